# revision 11
# baseline (speedup 1.0000x reference)
"""Multi-label masked-gather mean loss on 8 Trainium2 NeuronCores — v8.

loss = (sum_i logsumexp(x_i) + sum_{i,t} wneg[i,t]*x[i,y[i,t]]) / B

Hybrid sharding, all of x staged fp8 e4m3 (25.7 MB/core DMA):
 - ACT share (first AC columns): row-sharded. ScalarE Exp with
   in-instruction accumulate, 1 elem/cycle @ 1.2 GHz.
 - DVE share (last DC columns): COLUMN-sharded, host-transposed to
   xT [DC, B]; core m owns slab rows [m*DC/8, (m+1)*DC/8). On chip a
   tile is [128 partitions = 128 x-columns, free = all 4096 rows].
   VectorE computes pay_i16 = x*C0 + C1I (fp8->int16 tensor_scalar,
   2x mode, 0.5 cyc/elem; int16 value = bf16 bit pattern of e^x*2^K).
   TensorE reduces over columns: ones[128,1].T @ pay.bitcast(bf16)
   accumulated in PSUM [1, 4096] (8 banks) over all 30 chunks.
   PSUM is drained half by DVE, half by ScalarE (both see PSUM), so
   the tail copy is ~2.2us instead of 4.4.

v8 vs v7: the slab stream moved from gpsimd (SWDGE, ~1.6us per
dispatch — it starved the DVE) to the sync HWDGE ring shared with the
ACT stream; all load dispatches are emitted in estimated need-time
order so the single FIFO serves both consumers. DMA is the binding
resource (~75us at 341 GB/s); engines run at ~65-70us.

Host gathers the 8 labeled logits per row, pre-multiplies by -1/count
(device indirect gather broken in this environment, established in v4);
device reduces gw. DVE_BIAS removes the simulated residual
Schraudolph+fp8 bias of the payload path.
"""

import sys

sys.path.insert(0, "/opt/trn_rl_repo")

import numpy as np

import concourse.bass as bass
import concourse.tile as tile
from concourse import bacc, mybir
from concourse import bass_utils
from concourse.bass import MemorySpace

B, C, T = 4096, 50257, 8
NCORES = 8
BL = B // NCORES
P = 128
RB = BL // P
GCOLS = BL * T // P  # 32

_f32 = mybir.dt.float32
_bf16 = mybir.dt.bfloat16
_f8 = mybir.dt.float8e4
_i16 = mybir.dt.int16

# ---- column split ----
DC = 30720               # DVE/PE share, = 8 cores * 30 chunks * 128
AC = C - DC              # 19537, ACT share
DCC = DC // NCORES       # 3840 slab rows per core
NCHUNK = DCC // P        # 30 chunks of 128 x-columns
NBLK = B // 512          # 8 psum n-blocks of 512 rows

K_SCALE = 20.0
LOG2E = float(np.log2(np.e))
GAMMA = -0.0586
C0_SCH = LOG2E * 128.0
C1I_SCH = (127.0 + K_SCALE + GAMMA) * 128.0
DVE_BIAS = -1.6543e-04  # numpy sim of payload path on 67M N(0,1) samples


def _act_tiles(rb):
    if rb == 0:
        return [512, 1024, 2048, 4096, 5928, 5929]
    if rb == RB - 1:
        return [6512, 6513, 5012, 1200, 300]
    return [6512, 6512, 6513]


# chunk counts per DVE tile (each chunk = [128 cols, 4096 rows])
_DVE_TILE_CHUNKS = [1, 1] + [2] * 13 + [1, 1]
assert sum(_DVE_TILE_CHUNKS) == NCHUNK

for _rb in range(RB):
    assert sum(_act_tiles(_rb)) == AC

_NACT_BY_RB = [len(_act_tiles(rb)) for rb in range(RB)]
ACT_COLS_N = sum(_NACT_BY_RB)
MAXW_A = max(max(_act_tiles(rb)) for rb in range(RB)) + 1
MAXW_D = max(_DVE_TILE_CHUNKS) * B  # 8192

_compiled = None


def _events():
    """Merged (need_time_us, kind, payload) stream for both loads."""
    ev = []
    t = 0.0
    ca = 0
    for rb in range(RB):
        for i, wa in enumerate(_act_tiles(rb)):
            ev.append((t, "a", (rb, i, wa, ca)))
            ca += 1
            t += wa * 4 * 0.8333 / 4000.0  # us per tile (per-rowblock share)
    t = 0.0
    c0 = 0
    for ti, kc in enumerate(_DVE_TILE_CHUNKS):
        ev.append((t, "d", (ti, kc, c0)))
        c0 += kc
        t += kc * B * 0.46 / 1000.0
    ev.sort(key=lambda e: e[0])
    return ev


def _build():
    nc = bacc.Bacc(
        "TRN2",
        target_bir_lowering=False,
        debug=False,
        enable_asserts=False,
        num_devices=NCORES,
    )
    x_t = nc.dram_tensor("x", [BL, AC], _f8, kind="ExternalInput")
    xt_t = nc.dram_tensor("xt", [DCC, B], _f8, kind="ExternalInput")
    gw_t = nc.dram_tensor("gw", [P, GCOLS], _f32, kind="ExternalInput")
    outa_t = nc.dram_tensor("outa", [P, ACT_COLS_N], _f32, kind="ExternalOutput")
    outd_t = nc.dram_tensor("outd", [1, B], _f32, kind="ExternalOutput")
    outg_t = nc.dram_tensor("outg", [P, 1], _f32, kind="ExternalOutput")

    x = x_t.ap()
    xt = xt_t.ap()
    gw = gw_t.ap()
    outa = outa_t.ap()
    outd = outd_t.ap()
    outg = outg_t.ap()

    with tile.TileContext(nc) as tc:
        with (
            tc.tile_pool(name="scr4", bufs=1) as scr4_pool,
            tc.tile_pool(name="pay", bufs=2) as pay_pool,
            tc.tile_pool(name="din", bufs=4) as din_pool,
            tc.tile_pool(name="ain", bufs=4) as ain_pool,
            tc.tile_pool(name="scr8", bufs=1) as scr8_pool,
            tc.tile_pool(name="psum", bufs=1, space=MemorySpace.PSUM) as psum_pool,
        ):
            acc_a = scr4_pool.tile([P, ACT_COLS_N], _f32)
            ones = scr4_pool.tile([P, 1], _bf16)
            sum_sb = scr4_pool.tile([1, B], _f32)
            gw_tile = scr4_pool.tile([P, GCOLS], _f32)
            g_junk = scr4_pool.tile([P, GCOLS // 2], _f32)
            g_acc = scr4_pool.tile([P, 1], _f32)
            bias0 = scr4_pool.tile([P, 1], _f32)
            warm = scr4_pool.tile([P, 1], _f32)
            nc.gpsimd.memset(bias0[:], 0.0)
            nc.gpsimd.memset(ones[:], 1.0)

            exp_scr = scr8_pool.tile([P, MAXW_A], _f8)
            psum = psum_pool.tile([1, B], _f32)

            nc.scalar.dma_start(out=gw_tile[:], in_=gw[:])

            # warm the exp table during the first DMAs
            nc.scalar.activation(
                out=warm[:],
                in_=bias0[:, 0:1],
                func=mybir.ActivationFunctionType.Exp,
                bias=bias0[:, 0:1],
            )

            # gather dot on DVE, early
            nc.vector.scalar_tensor_tensor(
                out=g_junk[:],
                in0=gw_tile[:, : GCOLS // 2],
                scalar=1.0,
                in1=gw_tile[:, GCOLS // 2 :],
                op0=mybir.AluOpType.mult,
                op1=mybir.AluOpType.add,
                accum_out=g_acc[:],
            )
            nc.scalar.dma_start(out=outg[:], in_=g_acc[:])

            a0 = 0
            rb_prev = -1
            for ev_t, kind, pl in _events():
                if kind == "a":
                    rb, i, wa, ca = pl
                    if rb != rb_prev:
                        a0 = 0
                        rb_prev = rb
                    rows = slice(rb * P, (rb + 1) * P)
                    at = ain_pool.tile([P, MAXW_A], _f8, tag="at")
                    nc.sync.dma_start(out=at[:, :wa], in_=x[rows, a0 : a0 + wa])
                    nc.scalar.activation(
                        out=exp_scr[:, :wa],
                        in_=at[:, :wa],
                        func=mybir.ActivationFunctionType.Exp,
                        bias=bias0[:, 0:1],
                        accum_out=acc_a[:, ca : ca + 1],
                    )
                    a0 += wa
                else:
                    ti, kc, c0 = pl
                    w = kc * B
                    dt_ = din_pool.tile([P, MAXW_D], _f8, tag="dt")
                    pay = pay_pool.tile([P, MAXW_D], _i16, tag="pay")
                    for k in range(kc):
                        nc.sync.dma_start(
                            out=dt_[:, k * B : (k + 1) * B],
                            in_=xt[(c0 + k) * P : (c0 + k + 1) * P, :],
                        )
                    nc.vector.tensor_scalar(
                        pay[:, :w],
                        dt_[:, :w],
                        C0_SCH,
                        C1I_SCH,
                        mybir.AluOpType.mult,
                        mybir.AluOpType.add,
                    )
                    for k in range(kc):
                        c = c0 + k
                        for nb in range(NBLK):
                            nc.tensor.matmul(
                                psum[:, nb * 512 : (nb + 1) * 512],
                                ones[:],
                                pay[
                                    :, k * B + nb * 512 : k * B + (nb + 1) * 512
                                ].bitcast(_bf16),
                                start=(c == 0),
                                stop=(c == NCHUNK - 1),
                            )

            # drain PSUM row sums on DVE only — it finishes ~7us before the
            # ACT stream, so the full 4.3us copy hides under the final EXPs;
            # outputs dispatch from the idle sync ring, not behind the last EXP
            nc.vector.tensor_copy(out=sum_sb[:], in_=psum[:])
            nc.sync.dma_start(out=outd[:], in_=sum_sb[:])
            nc.sync.dma_start(out=outa[:], in_=acc_a[:])

    nc.compile()
    return nc


def _get_compiled():
    global _compiled
    if _compiled is None:
        _compiled = _build()
    return _compiled


def _make_in_maps(x, y):
    import ml_dtypes

    xf = np.asarray(x, dtype=np.float32)
    x8 = xf.astype(ml_dtypes.float8_e4m3)
    x8a = np.ascontiguousarray(x8[:, :AC])
    xt8 = np.ascontiguousarray(x8[:, AC:].T)  # [DC, B]
    y = np.asarray(y)
    mask = y != -1
    cnt = mask.sum(axis=1)
    w = np.where(mask, 1.0 / np.maximum(cnt, 1)[:, None], 0.0).astype(np.float32)
    safe = np.where(mask, y, 0)
    gvals = np.take_along_axis(
        xf.astype(ml_dtypes.bfloat16).astype(np.float32), safe, axis=1
    )
    gweighted = (gvals * np.where(mask, -w, 0.0)).astype(np.float32)

    in_maps = []
    for m in range(NCORES):
        sl = slice(m * BL, (m + 1) * BL)
        in_maps.append(
            {
                "x": x8a[sl],
                "xt": xt8[m * DCC : (m + 1) * DCC],
                "gw": np.ascontiguousarray(
                    gweighted[sl].reshape(P, GCOLS).astype(np.float32)
                ),
            }
        )
    return in_maps


def kernel(**inputs) -> np.ndarray:
    x, y = inputs["x"], inputs["y"]
    nc = _get_compiled()
    in_maps = _make_in_maps(x, y)
    res = bass_utils.run_bass_kernel_spmd(
        nc, in_maps, core_ids=list(range(NCORES))
    )
    # column-sharded DVE partial sums: add across cores -> [B]
    sd = np.zeros(B, dtype=np.float64)
    for r in res.results:
        sd += np.asarray(r["outd"], dtype=np.float64)[0]
    sd /= (2.0**K_SCALE) * (1.0 + DVE_BIAS)

    total = 0.0
    for m, r in enumerate(res.results):
        oa = np.asarray(r["outa"], dtype=np.float64)
        og = np.asarray(r["outg"], dtype=np.float64)
        ca = 0
        for rb in range(RB):
            na = _NACT_BY_RB[rb]
            rows = np.arange(m * BL + rb * P, m * BL + (rb + 1) * P)
            se = oa[:, ca : ca + na].sum(axis=1) + sd[rows]
            total += np.log(se).sum()
            ca += na
        total += og[:, 0].sum()
    return np.float32(total / B)
